# revision 15
# baseline (speedup 1.0000x reference)
"""Trainium2 Bass kernel for the BaseMemory coref scoring module.

Computes, for full inputs (M=65536 memory slots, D=768, E=20, H=64):
    score = relu(pair @ W1 + b1) @ W2 + b2, masked with ent_counter>0,
    where pair = [mem, ment, mem*ment, dist_emb, cnt_emb].

Sharding: data-parallel over the cluster dimension M across 8 NeuronCores.
Each core's shard of mem_vectors is laid out [D, MS] (contraction-major) so
the PE consumes it directly; all FLOPs and all HBM traffic stay on device.

Key folds (host side, O(D*H) + O(M) work on the small tensors only):
  - mem@W1_mem + (mem*ment)@W1_had = mem @ (W1_mem + diag(ment)@W1_had)
  - ment@W1_ment + b1 folded into the 10-row dist bucket table
  - bucket one-hots precomputed on host (O(M) int compares) and streamed
    as ONE [22, MS] fp16 plane; contracted on the PE against the folded
    10-row tables (masking folded into the PE accumulation, exact)
  - mem_vectors streamed as bf16: halves HBM traffic (the roofline term);
    all accumulation stays fp32 in PSUM

Scheduling (v5), from trace analysis:
  - SDMA bandwidth share between queues is ~proportional to descriptor
    size, so every x transfer is a contiguous-line block: A halves on
    the sync HWDGE ring, B halves on the scalar ring -> fair 50/50
    split in consumption order.
  - DMA issue #n blocks until DMA #(n-8) completes (8 completion
    lanes), so the kernel keeps few transfers, in consumption order.
  - The first and last pieces are additionally split along the
    contraction axis (k-chunks 0..2 / 3..5, still contiguous lines):
    the k0..k2 matmuls start ~2us earlier / overlap the final
    transfer, at zero extra PE cost.
  - w1 rides inside the first half-block as 64 extra columns per
    k-chunk (its own 768B-line DMA would be bandwidth-starved).
  - The score matmul is split into two accumulating matmuls
    (W2 x relu(z) + wsc_oh x onehot) so no per-pair one-hot staging
    tiles exist; scores trail the z pipeline by two pairs so a
    score matmul never blocks the PE FIFO waiting on a relu.
  - Each pair's z accumulation starts at k=0 (the bucket-table matmul
    closes the chain); start=True only on a region's first matmul
    (the has_written clear is strip-wide).
"""

import os
import numpy as np

# The bass kernel executes through the axon PJRT backend; make sure jax can
# see it even if the caller pinned JAX_PLATFORMS (e.g. to "cpu").
_jp = os.environ.get("JAX_PLATFORMS")
if _jp is not None and _jp != "" and "axon" not in _jp:
    os.environ["JAX_PLATFORMS"] = "axon," + _jp

M, D, E, H = 65536, 768, 20, 64
N_CORES = 8
MS = M // N_CORES          # rows per core = 8192
GROUP = 512                # rows per PE matmul group
N_GROUPS = MS // GROUP     # 16
SG = 4                     # groups per output super-group
N_SG = N_GROUPS // SG      # 4
KCH = D // 128             # 6 contraction chunks
KSP = 3                    # k-split boundary for first/last pieces
NF = 22                    # 10 dist onehot, 10 cnt onehot, notmask, ones
NPAIR = N_GROUPS // 2      # 8 column-pair blocks per core
PB = 2 * GROUP             # 1024 columns per pair block
BIG = float(2 ** 14)       # pre-relu kill value for masked rows (fp16-exact)

_CACHE = {}


def _build():
    """Build + compile the 8-core SPMD bass program once per process."""
    if "nc" in _CACHE:
        return _CACHE["nc"]

    import concourse.bass as bass
    import concourse.mybir as mybir
    import concourse.tile as tile
    from concourse import bacc

    F32 = mybir.dt.float32
    BF16 = mybir.dt.bfloat16
    FP16 = mybir.dt.float16

    nc = bacc.Bacc("TRN2", target_bir_lowering=False, debug=False,
                   enable_asserts=False, num_devices=N_CORES)

    # x pre-tiled on host as contiguous half-blocks [hb, partition,
    # kchunk, col]: each DMA moves one half-block with a single 6KB
    # contiguous line per partition.  Half-block 0 carries w1 as 64
    # extra columns per k-chunk.
    x0_d = nc.dram_tensor("x0", [128, KCH, GROUP + H], BF16,
                          kind="ExternalInput").ap()
    xt_d = nc.dram_tensor("xt", [2 * NPAIR - 1, 128, KCH, GROUP], BF16,
                          kind="ExternalInput").ap()
    oh_d = nc.dram_tensor("oh", [NF, MS], FP16, kind="ExternalInput").ap()
    # packed small consts: cols 0..63 rows 0..21 = folded bucket tables,
    # col 64 = W2, col 65 rows 20/21 = mask/bias score weights
    p_d = nc.dram_tensor("pk", [H, 66], FP16, kind="ExternalInput").ap()
    out_d = nc.dram_tensor("out", [MS], F32, kind="ExternalOutput").ap()

    out_r = out_d.rearrange("(s c) -> s c", s=N_SG)     # [4, 2048]

    relu = mybir.ActivationFunctionType.Relu

    with tile.TileContext(nc) as tc:
        with (
            tc.tile_pool(name="consts", bufs=1) as cpool,
            tc.tile_pool(name="xin", bufs=15) as px,
            tc.tile_pool(name="hts", bufs=8) as ph,
            tc.tile_pool(name="osb", bufs=2) as posb,
            tc.tile_pool(name="psz", bufs=4, space="PSUM") as psz,
            tc.tile_pool(name="pss", bufs=4, space="PSUM") as pss,
        ):
            # loads in consumption order.  The first three pairs' halves
            # (and the final half) are k-split ACROSS the rings -- k0..2
            # on sync, k3..5 on scalar -- so each chain's sub-pieces
            # arrive concurrently at 2x the per-ring rate, in exact
            # consumption order (the early PE gaps were serial per-ring
            # half-block waits).  Mid-stream pairs stay whole (the PE
            # runs behind the stream there); oh sits early on the scalar
            # ring (needed by the chain-closing bucket matmuls from
            # ~15us); pk rides the idle gpsimd SWDGE queue.
            pk = cpool.tile([H, 66], FP16, tag="pk")
            nc.gpsimd.dma_start(pk[:], p_d[:])
            ohb = cpool.tile([NF, MS], FP16, tag="ohb")

            halves = []
            for hb in range(2 * NPAIR):
                if hb == 0:
                    xh = cpool.tile([128, KCH, GROUP + H], BF16, tag="x0")
                    src_ap = x0_d
                else:
                    xh = px.tile([128, KCH, GROUP], BF16, tag="xin")
                    src_ap = xt_d[hb - 1]
                if hb < 6 or hb == 2 * NPAIR - 1:
                    nc.sync.dma_start(xh[:, 0:KSP, :], src_ap[:, 0:KSP, :])
                    nc.scalar.dma_start(xh[:, KSP:KCH, :],
                                        src_ap[:, KSP:KCH, :])
                else:
                    eng = nc.sync if hb % 2 == 0 else nc.scalar
                    eng.dma_start(xh[:], src_ap)
                halves.append(xh)
                if hb == 1:
                    nc.scalar.dma_start(ohb[:], oh_d[:])
            x0 = halves[0]

            def w1(k):
                return x0[:, k, GROUP:GROUP + H]

            osb_tiles = {}
            from collections import deque
            pending = deque()

            def emit_score(g, hq, hoff):
                # per-group score: two accumulating matmuls into one
                # 1-bank PSUM tile (W2 x relu(z), then wsc_oh x onehot)
                sc = pss.tile([1, GROUP], F32, tag="pss")
                nc.tensor.matmul(sc[:], pk[0:H, 64:65],
                                 hq[0:H, hoff:hoff + GROUP],
                                 start=True, stop=False,
                                 skip_group_check=True)
                nc.tensor.matmul(sc[:], pk[0:NF, 65:66],
                                 ohb[:, GROUP * g:GROUP * (g + 1)],
                                 start=False, stop=True,
                                 skip_group_check=True)
                sq, j = divmod(g, SG)
                if j == 0:
                    osb_new = posb.tile([1, SG * GROUP], F32, tag="osb")
                    osb_tiles[sq] = osb_new
                orow = osb_tiles[sq][0:1, GROUP * j:GROUP * (j + 1)]
                # odd groups copy on ACT so the final (odd) group's copy
                # never queues behind the final relu on DVE
                if g % 2 == 0:
                    nc.vector.tensor_copy(orow, sc[:])
                else:
                    nc.scalar.copy(orow, sc[:])
                last = sq == N_SG - 1
                if last and j == SG - 2:
                    # ship the last super-group's first 3 groups early so
                    # only one small store trails the final score
                    nc.gpsimd.dma_start(
                        out_r[sq:sq + 1, 0:GROUP * (SG - 1)],
                        osb_tiles[sq][0:1, 0:GROUP * (SG - 1)])
                if j == SG - 1:
                    # the final store rides the by-then idle sync HWDGE
                    # ring (lower fixed latency than SWDGE)
                    if last:
                        nc.sync.dma_start(
                            out_r[sq:sq + 1, GROUP * (SG - 1):],
                            osb_tiles.pop(sq)[0:1, GROUP * (SG - 1):])
                    else:
                        nc.gpsimd.dma_start(out_r[sq:sq + 1, :],
                                            osb_tiles.pop(sq)[:])

            for q in range(NPAIR):
                xa = halves[2 * q]
                xb = halves[2 * q + 1]
                # two groups per PSUM tile, computed on the two PE column
                # halves (tile_position inferred from the output base
                # partition)
                zt = psz.tile([2 * H, GROUP], F32, tag="psz")
                if q == NPAIR - 1:
                    # separate PSUM tile for the last pair's B group, so
                    # relu-A's (tile-granular) dependency does not include
                    # the B matmuls that wait on the final k-split DMA
                    ztb = psz.tile([2 * H, GROUP], F32, tag="psz")
                else:
                    ztb = zt
                # z chains open on k=0 so the PE starts on the first x
                # bytes; the bucket-table matmul (deps: ohb+pk only)
                # closes each chain
                if q == NPAIR - 1:
                    for k in range(KCH):
                        nc.tensor.matmul(zt[0:H, :], w1(k), xa[:, k, 0:GROUP],
                                         start=(k == 0), stop=False,
                                         skip_group_check=True)
                    for k in range(KCH):
                        nc.tensor.matmul(ztb[H:2 * H, :], w1(k),
                                         xb[:, k, 0:GROUP],
                                         start=(k == 0), stop=False,
                                         skip_group_check=True)
                else:
                    for k in range(KCH):
                        nc.tensor.matmul(zt[0:H, :], w1(k), xa[:, k, 0:GROUP],
                                         start=(k == 0), stop=False,
                                         skip_group_check=True)
                        nc.tensor.matmul(ztb[H:2 * H, :], w1(k),
                                         xb[:, k, 0:GROUP],
                                         start=(k == 0), stop=False,
                                         skip_group_check=True)
                nc.tensor.matmul(zt[0:H, :], pk[0:NF, 0:H],
                                 ohb[:, PB * q:PB * q + GROUP],
                                 start=False, stop=True,
                                 skip_group_check=True)
                nc.tensor.matmul(ztb[H:2 * H, :], pk[0:NF, 0:H],
                                 ohb[:, PB * q + GROUP:PB * (q + 1)],
                                 start=False, stop=True,
                                 skip_group_check=True)

                # relus run concurrently on ACT and DVE
                hq = ph.tile([H, PB], FP16, tag="hts")
                nc.scalar.activation(hq[:, 0:GROUP], zt[0:H, :], relu)
                nc.vector.tensor_scalar_max(hq[:, GROUP:PB],
                                            ztb[H:2 * H, :], 0.0)
                pending.append((2 * q, hq, 0))
                pending.append((2 * q + 1, hq, GROUP))
                # scores trail by two pairs so they never stall the PE
                # queue waiting on a relu
                while len(pending) >= 5:
                    emit_score(*pending.popleft())
            while pending:
                emit_score(*pending.popleft())

    nc.compile()
    _CACHE["nc"] = nc
    return nc


def _bucket(c):
    """Reference get_bucket, replicated with the same XLA CPU float ops so
    boundary cases (c = 8, 16, 32) bucket identically."""
    import math
    import jax
    import jax.numpy as jnp
    cpu = jax.devices("cpu")[0]
    with jax.default_device(cpu):
        c = jnp.asarray(c).astype(jnp.int32)
        logspace = jnp.floor(
            jnp.log(jnp.maximum(c, 1).astype(jnp.float32)) / math.log(2)
        ).astype(jnp.int32) + 3
        idx = jnp.where(c <= 4, c, logspace)
        return np.asarray(jnp.clip(idx, 0, 9))


def _prepare_maps(ment_emb, mem_vectors, dist_table, counter_table,
                  W1, b1, W2, b2, ent_counter, last_mention_start, ment_start):
    import ml_dtypes
    f32 = np.float32
    bf16 = ml_dtypes.bfloat16
    fp16 = np.float16
    ment = np.asarray(ment_emb, f32)
    mem = np.asarray(mem_vectors, f32)
    W1 = np.asarray(W1, f32)
    ms = int(np.asarray(ment_start))

    W1m, W1r, W1h = W1[0:D], W1[D:2 * D], W1[2 * D:3 * D]
    W1d, W1c = W1[3 * D:3 * D + E], W1[3 * D + E:3 * D + 2 * E]

    w1eff = (W1m + ment[:, None] * W1h).astype(f32)              # [768, 64]
    bias_vec = (np.asarray(b1, f32) + ment @ W1r).astype(f32)    # [64]
    T_d = (np.asarray(dist_table, f32) @ W1d + bias_vec).astype(f32)
    T_c = (np.asarray(counter_table, f32) @ W1c).astype(f32)
    b2v = float(np.asarray(b2, f32).reshape(-1)[0])

    # pk: cols 0..63 = folded bucket tables (rows 0..9 dist incl. bias,
    # 10..19 counter, 20 = -BIG mask kill, 21 unused); col 64 = W2;
    # col 65 = onehot-side score weights (mask -10000 and bias b2)
    pk = np.zeros((H, 66), f32)
    pk[0:10, 0:H] = T_d
    pk[10:20, 0:H] = T_c
    pk[20, 0:H] = -BIG
    pk[0:H, 64] = np.asarray(W2, f32).reshape(-1)
    pk[20, 65] = -10000.0 - b2v
    pk[21, 65] = b2v

    cnt_i = np.asarray(ent_counter).astype(np.int64)
    dist_i = ms - np.asarray(last_mention_start).astype(np.int64)
    bd = _bucket(dist_i)                                         # [M] in 0..9
    bc = _bucket(cnt_i)                                          # [M] in 0..9
    r = np.arange(10)
    oh = np.empty((NF, M), f32)
    oh[0:10] = (bd[None, :] == r[:, None])
    oh[10:20] = (bc[None, :] == r[:, None])
    oh[20] = (cnt_i <= 0)
    oh[21] = 1.0
    oh = oh.astype(fp16)

    # w1 tiled [128, kchunk, 64] to ride inside the first half-block
    w1_t = (w1eff.reshape(KCH, 128, H).transpose(1, 0, 2)).astype(bf16)
    pk_b = pk.astype(fp16)

    in_maps = []
    for c in range(N_CORES):
        sl = slice(c * MS, (c + 1) * MS)
        # [hb, partition, kchunk, col] pre-tiling: one contiguous 6KB
        # line per partition per half-block DMA
        xt = (mem[sl].T.astype(bf16)
              .reshape(KCH, 128, 2 * NPAIR, GROUP)
              .transpose(2, 1, 0, 3))
        x0 = np.concatenate([xt[0], w1_t], axis=2)   # [128, KCH, 576]
        in_maps.append(dict(
            x0=np.ascontiguousarray(x0),
            xt=np.ascontiguousarray(xt[1:]),
            oh=np.ascontiguousarray(oh[:, sl]),
            pk=pk_b))
    return in_maps


def _postprocess(results):
    out = np.empty(M + 1, np.float32)
    for c in range(N_CORES):
        out[c * MS:(c + 1) * MS] = results[c]["out"]
    out[M] = 0.0
    return out


def run_spmd(in_maps, trace=False):
    from concourse.bass_utils import run_bass_kernel_spmd
    nc = _build()
    return run_bass_kernel_spmd(nc, in_maps, list(range(N_CORES)), trace=trace)


def kernel(**inputs):
    in_maps = _prepare_maps(**inputs)
    res = run_spmd(in_maps, trace=False)
    return _postprocess(res.results)


# revision 16
# speedup vs baseline: 1.1167x; 1.1167x over previous
"""Trainium2 Bass kernel for the BaseMemory coref scoring module.

Computes, for full inputs (M=65536 memory slots, D=768, E=20, H=64):
    score = relu(pair @ W1 + b1) @ W2 + b2, masked with ent_counter>0,
    where pair = [mem, ment, mem*ment, dist_emb, cnt_emb].

Sharding: data-parallel over the cluster dimension M across 8 NeuronCores.
Each core's shard of mem_vectors is laid out [D, MS] (contraction-major) so
the PE consumes it directly; all FLOPs and all HBM traffic stay on device.

Key folds (host side, O(D*H) + O(M) work on the small tensors only):
  - mem@W1_mem + (mem*ment)@W1_had = mem @ (W1_mem + diag(ment)@W1_had)
  - ment@W1_ment + b1 folded into the 10-row dist bucket table
  - bucket one-hots precomputed on host (O(M) int compares) and streamed
    as ONE [22, MS] fp16 plane; contracted on the PE against the folded
    10-row tables (masking folded into the PE accumulation, exact)
  - mem_vectors streamed as bf16: halves HBM traffic (the roofline term);
    all accumulation stays fp32 in PSUM

Scheduling (v5), from trace analysis:
  - SDMA bandwidth share between queues is ~proportional to descriptor
    size, so every x transfer is a contiguous-line block: A halves on
    the sync HWDGE ring, B halves on the scalar ring -> fair 50/50
    split in consumption order.
  - DMA issue #n blocks until DMA #(n-8) completes (8 completion
    lanes), so the kernel keeps few transfers, in consumption order.
  - The first and last pieces are additionally split along the
    contraction axis (k-chunks 0..2 / 3..5, still contiguous lines):
    the k0..k2 matmuls start ~2us earlier / overlap the final
    transfer, at zero extra PE cost.
  - w1 rides inside the first half-block as 64 extra columns per
    k-chunk (its own 768B-line DMA would be bandwidth-starved).
  - The score matmul is split into two accumulating matmuls
    (W2 x relu(z) + wsc_oh x onehot) so no per-pair one-hot staging
    tiles exist; scores trail the z pipeline by two pairs so a
    score matmul never blocks the PE FIFO waiting on a relu.
  - Each pair's z accumulation starts at k=0 (the bucket-table matmul
    closes the chain); start=True only on a region's first matmul
    (the has_written clear is strip-wide).
"""

import os
import numpy as np

# The bass kernel executes through the axon PJRT backend; make sure jax can
# see it even if the caller pinned JAX_PLATFORMS (e.g. to "cpu").
_jp = os.environ.get("JAX_PLATFORMS")
if _jp is not None and _jp != "" and "axon" not in _jp:
    os.environ["JAX_PLATFORMS"] = "axon," + _jp

M, D, E, H = 65536, 768, 20, 64
N_CORES = 8
MS = M // N_CORES          # rows per core = 8192
GROUP = 512                # rows per PE matmul group
N_GROUPS = MS // GROUP     # 16
SG = 4                     # groups per output super-group
N_SG = N_GROUPS // SG      # 4
KCH = D // 128             # 6 contraction chunks
KSP = 3                    # k-split boundary for first/last pieces
NF = 22                    # 10 dist onehot, 10 cnt onehot, notmask, ones
NPAIR = N_GROUPS // 2      # 8 column-pair blocks per core
PB = 2 * GROUP             # 1024 columns per pair block
BIG = float(2 ** 14)       # pre-relu kill value for masked rows (fp16-exact)

_CACHE = {}


def _build():
    """Build + compile the 8-core SPMD bass program once per process."""
    if "nc" in _CACHE:
        return _CACHE["nc"]

    import concourse.bass as bass
    import concourse.mybir as mybir
    import concourse.tile as tile
    from concourse import bacc

    F32 = mybir.dt.float32
    BF16 = mybir.dt.bfloat16
    FP16 = mybir.dt.float16

    nc = bacc.Bacc("TRN2", target_bir_lowering=False, debug=False,
                   enable_asserts=False, num_devices=N_CORES)

    # x pre-tiled on host as contiguous half-blocks [hb, partition,
    # kchunk, col]: each DMA moves one half-block with a single 6KB
    # contiguous line per partition.  Half-block 0 carries w1 as 64
    # extra columns per k-chunk.
    x0_d = nc.dram_tensor("x0", [128, KCH, GROUP + H], BF16,
                          kind="ExternalInput").ap()
    xt_d = nc.dram_tensor("xt", [2 * NPAIR - 1, 128, KCH, GROUP], BF16,
                          kind="ExternalInput").ap()
    oh_d = nc.dram_tensor("oh", [NF, MS], FP16, kind="ExternalInput").ap()
    # packed small consts: cols 0..63 rows 0..21 = folded bucket tables,
    # col 64 = W2, col 65 rows 20/21 = mask/bias score weights
    p_d = nc.dram_tensor("pk", [H, 66], FP16, kind="ExternalInput").ap()
    out_d = nc.dram_tensor("out", [MS], F32, kind="ExternalOutput").ap()

    out_r = out_d.rearrange("(s c) -> s c", s=N_SG)     # [4, 2048]

    relu = mybir.ActivationFunctionType.Relu

    with tile.TileContext(nc) as tc:
        with (
            tc.tile_pool(name="consts", bufs=1) as cpool,
            tc.tile_pool(name="xin", bufs=15) as px,
            tc.tile_pool(name="hts", bufs=8) as ph,
            tc.tile_pool(name="osb", bufs=2) as posb,
            tc.tile_pool(name="psz", bufs=4, space="PSUM") as psz,
            tc.tile_pool(name="pss", bufs=4, space="PSUM") as pss,
        ):
            # loads in consumption order: A halves (+w1 in the first) on
            # the sync ring, B halves then oh on the scalar ring (oh is
            # only needed by the chain-closing bucket matmuls); pk rides
            # the otherwise idle gpsimd SWDGE queue.  First/last pieces
            # are k-split so their k0..2 matmuls overlap the k3..5 DMA.
            x0 = cpool.tile([128, KCH, GROUP + H], BF16, tag="x0")
            nc.sync.dma_start(x0[:, 0:KSP, :], x0_d[:, 0:KSP, :])
            nc.sync.dma_start(x0[:, KSP:KCH, :], x0_d[:, KSP:KCH, :])
            pk = cpool.tile([H, 66], FP16, tag="pk")
            nc.gpsimd.dma_start(pk[:], p_d[:])

            x0b = px.tile([128, KCH, GROUP], BF16, tag="xin")
            nc.scalar.dma_start(x0b[:, 0:KSP, :], xt_d[0][:, 0:KSP, :])
            nc.scalar.dma_start(x0b[:, KSP:KCH, :], xt_d[0][:, KSP:KCH, :])
            ohb = cpool.tile([NF, MS], FP16, tag="ohb")
            nc.scalar.dma_start(ohb[:], oh_d[:])

            halves = [x0, x0b]
            for hb in range(2, 2 * NPAIR):
                xh = px.tile([128, KCH, GROUP], BF16, tag="xin")
                eng = nc.sync if hb % 2 == 0 else nc.scalar
                if hb == 2 * NPAIR - 1:
                    # final piece k-split so only k3..5 trail the last byte
                    eng.dma_start(xh[:, 0:KSP, :], xt_d[hb - 1][:, 0:KSP, :])
                    eng.dma_start(xh[:, KSP:KCH, :],
                                  xt_d[hb - 1][:, KSP:KCH, :])
                else:
                    eng.dma_start(xh[:], xt_d[hb - 1])
                halves.append(xh)

            def w1(k):
                return x0[:, k, GROUP:GROUP + H]

            osb_tiles = {}
            from collections import deque
            pending = deque()

            def emit_score(g, hq, hoff):
                # per-group score: two accumulating matmuls into one
                # 1-bank PSUM tile (W2 x relu(z), then wsc_oh x onehot)
                sc = pss.tile([1, GROUP], F32, tag="pss")
                nc.tensor.matmul(sc[:], pk[0:H, 64:65],
                                 hq[0:H, hoff:hoff + GROUP],
                                 start=True, stop=False,
                                 skip_group_check=True)
                nc.tensor.matmul(sc[:], pk[0:NF, 65:66],
                                 ohb[:, GROUP * g:GROUP * (g + 1)],
                                 start=False, stop=True,
                                 skip_group_check=True)
                sq, j = divmod(g, SG)
                if j == 0:
                    osb_new = posb.tile([1, SG * GROUP], F32, tag="osb")
                    osb_tiles[sq] = osb_new
                orow = osb_tiles[sq][0:1, GROUP * j:GROUP * (j + 1)]
                # odd groups copy on ACT so the final (odd) group's copy
                # never queues behind the final relu on DVE
                if g % 2 == 0:
                    nc.vector.tensor_copy(orow, sc[:])
                else:
                    nc.scalar.copy(orow, sc[:])
                last = sq == N_SG - 1
                if last and j == SG - 2:
                    # ship the last super-group's first 3 groups early so
                    # only one small store trails the final score
                    nc.gpsimd.dma_start(
                        out_r[sq:sq + 1, 0:GROUP * (SG - 1)],
                        osb_tiles[sq][0:1, 0:GROUP * (SG - 1)])
                if j == SG - 1:
                    # the final store rides the by-then idle sync HWDGE
                    # ring (lower fixed latency than SWDGE)
                    if last:
                        nc.sync.dma_start(
                            out_r[sq:sq + 1, GROUP * (SG - 1):],
                            osb_tiles.pop(sq)[0:1, GROUP * (SG - 1):])
                    else:
                        nc.gpsimd.dma_start(out_r[sq:sq + 1, :],
                                            osb_tiles.pop(sq)[:])

            for q in range(NPAIR):
                xa = halves[2 * q]
                xb = halves[2 * q + 1]
                # two groups per PSUM tile, computed on the two PE column
                # halves (tile_position inferred from the output base
                # partition)
                zt = psz.tile([2 * H, GROUP], F32, tag="psz")
                if q == NPAIR - 1:
                    # separate PSUM tile for the last pair's B group, so
                    # relu-A's (tile-granular) dependency does not include
                    # the B matmuls that wait on the final k-split DMA
                    ztb = psz.tile([2 * H, GROUP], F32, tag="psz")
                else:
                    ztb = zt
                # z chains open on k=0 so the PE starts on the first x
                # bytes; the bucket-table matmul (deps: ohb+pk only)
                # closes each chain
                if q == NPAIR - 1:
                    for k in range(KCH):
                        nc.tensor.matmul(zt[0:H, :], w1(k), xa[:, k, 0:GROUP],
                                         start=(k == 0), stop=False,
                                         skip_group_check=True)
                    for k in range(KCH):
                        nc.tensor.matmul(ztb[H:2 * H, :], w1(k),
                                         xb[:, k, 0:GROUP],
                                         start=(k == 0), stop=False,
                                         skip_group_check=True)
                else:
                    for k in range(KCH):
                        nc.tensor.matmul(zt[0:H, :], w1(k), xa[:, k, 0:GROUP],
                                         start=(k == 0), stop=False,
                                         skip_group_check=True)
                        nc.tensor.matmul(ztb[H:2 * H, :], w1(k),
                                         xb[:, k, 0:GROUP],
                                         start=(k == 0), stop=False,
                                         skip_group_check=True)
                nc.tensor.matmul(zt[0:H, :], pk[0:NF, 0:H],
                                 ohb[:, PB * q:PB * q + GROUP],
                                 start=False, stop=True,
                                 skip_group_check=True)
                nc.tensor.matmul(ztb[H:2 * H, :], pk[0:NF, 0:H],
                                 ohb[:, PB * q + GROUP:PB * (q + 1)],
                                 start=False, stop=True,
                                 skip_group_check=True)

                # relus run concurrently on ACT and DVE
                hq = ph.tile([H, PB], FP16, tag="hts")
                nc.scalar.activation(hq[:, 0:GROUP], zt[0:H, :], relu)
                nc.vector.tensor_scalar_max(hq[:, GROUP:PB],
                                            ztb[H:2 * H, :], 0.0)
                pending.append((2 * q, hq, 0))
                pending.append((2 * q + 1, hq, GROUP))
                # scores trail by two pairs so they never stall the PE
                # queue waiting on a relu
                while len(pending) >= 5:
                    emit_score(*pending.popleft())
            while pending:
                emit_score(*pending.popleft())

    nc.compile()
    _CACHE["nc"] = nc
    return nc


def _bucket(c):
    """Reference get_bucket, replicated with the same XLA CPU float ops so
    boundary cases (c = 8, 16, 32) bucket identically."""
    import math
    import jax
    import jax.numpy as jnp
    cpu = jax.devices("cpu")[0]
    with jax.default_device(cpu):
        c = jnp.asarray(c).astype(jnp.int32)
        logspace = jnp.floor(
            jnp.log(jnp.maximum(c, 1).astype(jnp.float32)) / math.log(2)
        ).astype(jnp.int32) + 3
        idx = jnp.where(c <= 4, c, logspace)
        return np.asarray(jnp.clip(idx, 0, 9))


def _prepare_maps(ment_emb, mem_vectors, dist_table, counter_table,
                  W1, b1, W2, b2, ent_counter, last_mention_start, ment_start):
    import ml_dtypes
    f32 = np.float32
    bf16 = ml_dtypes.bfloat16
    fp16 = np.float16
    ment = np.asarray(ment_emb, f32)
    mem = np.asarray(mem_vectors, f32)
    W1 = np.asarray(W1, f32)
    ms = int(np.asarray(ment_start))

    W1m, W1r, W1h = W1[0:D], W1[D:2 * D], W1[2 * D:3 * D]
    W1d, W1c = W1[3 * D:3 * D + E], W1[3 * D + E:3 * D + 2 * E]

    w1eff = (W1m + ment[:, None] * W1h).astype(f32)              # [768, 64]
    bias_vec = (np.asarray(b1, f32) + ment @ W1r).astype(f32)    # [64]
    T_d = (np.asarray(dist_table, f32) @ W1d + bias_vec).astype(f32)
    T_c = (np.asarray(counter_table, f32) @ W1c).astype(f32)
    b2v = float(np.asarray(b2, f32).reshape(-1)[0])

    # pk: cols 0..63 = folded bucket tables (rows 0..9 dist incl. bias,
    # 10..19 counter, 20 = -BIG mask kill, 21 unused); col 64 = W2;
    # col 65 = onehot-side score weights (mask -10000 and bias b2)
    pk = np.zeros((H, 66), f32)
    pk[0:10, 0:H] = T_d
    pk[10:20, 0:H] = T_c
    pk[20, 0:H] = -BIG
    pk[0:H, 64] = np.asarray(W2, f32).reshape(-1)
    pk[20, 65] = -10000.0 - b2v
    pk[21, 65] = b2v

    cnt_i = np.asarray(ent_counter).astype(np.int64)
    dist_i = ms - np.asarray(last_mention_start).astype(np.int64)
    bd = _bucket(dist_i)                                         # [M] in 0..9
    bc = _bucket(cnt_i)                                          # [M] in 0..9
    r = np.arange(10)
    oh = np.empty((NF, M), f32)
    oh[0:10] = (bd[None, :] == r[:, None])
    oh[10:20] = (bc[None, :] == r[:, None])
    oh[20] = (cnt_i <= 0)
    oh[21] = 1.0
    oh = oh.astype(fp16)

    # w1 tiled [128, kchunk, 64] to ride inside the first half-block
    w1_t = (w1eff.reshape(KCH, 128, H).transpose(1, 0, 2)).astype(bf16)
    pk_b = pk.astype(fp16)

    in_maps = []
    for c in range(N_CORES):
        sl = slice(c * MS, (c + 1) * MS)
        # [hb, partition, kchunk, col] pre-tiling: one contiguous 6KB
        # line per partition per half-block DMA
        xt = (mem[sl].T.astype(bf16)
              .reshape(KCH, 128, 2 * NPAIR, GROUP)
              .transpose(2, 1, 0, 3))
        x0 = np.concatenate([xt[0], w1_t], axis=2)   # [128, KCH, 576]
        in_maps.append(dict(
            x0=np.ascontiguousarray(x0),
            xt=np.ascontiguousarray(xt[1:]),
            oh=np.ascontiguousarray(oh[:, sl]),
            pk=pk_b))
    return in_maps


def _postprocess(results):
    out = np.empty(M + 1, np.float32)
    for c in range(N_CORES):
        out[c * MS:(c + 1) * MS] = results[c]["out"]
    out[M] = 0.0
    return out


def run_spmd(in_maps, trace=False):
    from concourse.bass_utils import run_bass_kernel_spmd
    nc = _build()
    return run_bass_kernel_spmd(nc, in_maps, list(range(N_CORES)), trace=trace)


def kernel(**inputs):
    in_maps = _prepare_maps(**inputs)
    res = run_spmd(in_maps, trace=False)
    return _postprocess(res.results)
